# revision 5
# baseline (speedup 1.0000x reference)
"""Additive-attention pooling (nn_Meta_Module) Trainium2 kernel.

Full inputs in, full output out. Internally: pure data-parallel over 8
NeuronCores (batch 512 -> 64/core). Per core, a Bass/Tile kernel computes
  a   = all_memory @ U.T            (PE, bf16, [k,(b,s)] layout)
  t   = tanh(a + last @ W.T)        (ScalarE, per-batch bias)
  sc  = V.T @ t                     (PE, V-stationary)
  P   = all_memory @ MetaW.T        (PE, col-tiled partition stack)
  out = (sum_s e^sc * P) / sum_s e^sc + Metab   (PE selector + DVE, host divide)
"""
import numpy as np
import ml_dtypes
from contextlib import ExitStack

import concourse.bass as bass
import concourse.tile as tile
import concourse.mybir as mybir
from concourse import bacc
from concourse.bass_utils import run_bass_kernel_spmd

BF16 = mybir.dt.bfloat16
F32 = mybir.dt.float32
AF = mybir.ActivationFunctionType
ALU = mybir.AluOpType
NBF = ml_dtypes.bfloat16

B, S, H = 512, 200, 256
N_CORES = 8
B_LOC = B // N_CORES


def build_nc(b_loc=B_LOC, debug=False):
    GROUP = b_loc // 2
    ROUNDS = b_loc // 4
    PROWS = 32 * ((GROUP + 7) // 8)
    nc = bacc.Bacc("TRN2", target_bir_lowering=False, debug=debug)

    def din(name, shape, dt=BF16):
        return nc.dram_tensor(name, shape, dt, kind="ExternalInput")

    allT = [din(f"allT{h}", [128, b_loc * S]) for h in range(2)]
    CBa_d = din("CBa", [128, 512])
    CBb_d = din("CBb", [128, 2 + 512 + 128])
    LT_d = din("LT", [128, 2 * b_loc], F32)
    numer_d = nc.dram_tensor("numer", [PROWS, 2], F32, kind="ExternalOutput")
    esum_d = nc.dram_tensor("esum", [b_loc, 1], F32, kind="ExternalOutput")

    with tile.TileContext(nc) as tc, ExitStack() as ctx:
        consts = ctx.enter_context(tc.tile_pool(name="consts", bufs=1))
        allp = ctx.enter_context(tc.tile_pool(name="allp", bufs=12))
        tpool = ctx.enter_context(tc.tile_pool(name="tpool", bufs=14))
        misc = ctx.enter_context(tc.tile_pool(name="misc", bufs=2))
        pa = ctx.enter_context(tc.tile_pool(name="pa", bufs=7, space="PSUM"))
        ps_pt = ctx.enter_context(tc.tile_pool(name="ps_pt", bufs=1, space="PSUM"))

        def load_const(dram, shape, tag, dt=BF16):
            t = consts.tile(shape, dt, tag=tag)
            nc.scalar.dma_start(t[:], dram.ap())
            return t

        cba = load_const(CBa_d, [128, 512], "c_cba")
        lt = load_const(LT_d, [128, 2 * b_loc], "c_lt", F32)
        cbb = load_const(CBb_d, [128, 2 + 512 + 128], "c_cbb")
        ut = cba[:, 0:512]
        vt = cbb[:, 0:2]
        mwp = cbb[:, 2:514]
        sel4 = cbb[0:GROUP, 514:642]

        def UT_ap(h, k):
            return ut[:, (2 * h + k) * 128:(2 * h + k + 1) * 128]

        def MW_ap(h, i):
            return mwp[:, (h * 8 + i) * 32:(h * 8 + i) * 32 + 32]

        scores_flat = consts.tile([1, b_loc * S], F32, tag="c_scf")
        PT = ps_pt.tile([PROWS, 2 * S], F32)
        pt_blocks = set()

        def emit_tail(r, at, tts):
            for u in range(2):
                sc = pa.tile([1, 2 * S], F32, tag="pa")
                for k in range(2):
                    nc.tensor.matmul(sc[:], vt[:, k:k + 1], tts[(u, k)][:],
                                     start=(k == 0), stop=(k == 1))
                nc.vector.tensor_copy(
                    scores_flat[0:1, (4 * r + 2 * u) * S:(4 * r + 2 * u + 2) * S],
                    sc[:])
            for u in range(2):
                for bb in range(2):
                    b = 4 * r + 2 * u + bb
                    bl = b % GROUP
                    g2 = b // GROUP
                    j32 = 32 * (bl // 8)
                    i8 = bl % 8
                    blk_new = (g2, j32) not in pt_blocks
                    pt_blocks.add((g2, j32))
                    for h in range(2):
                        nc.tensor.matmul(
                            PT[j32:j32 + 32, g2 * S:(g2 + 1) * S],
                            MW_ap(h, i8),
                            at[h][:, (2 * u + bb) * S:(2 * u + bb + 1) * S],
                            tile_position=(0, j32),
                            start=(blk_new and h == 0), stop=(h == 1),
                            skip_group_check=True)

        numer = misc.tile([PROWS, 2], F32, tag="numer", bufs=1)

        def endgame(g2):
            g0 = g2 * GROUP
            scT_g = misc.tile([GROUP, S], F32, tag="scT")
            nc.sync.dma_start(scT_g[:],
                              scores_flat[0:1, g0 * S:(g0 + GROUP) * S])
            e_g = misc.tile([GROUP, S], BF16, tag="e")
            nc.scalar.activation(e_g[:], scT_g[:], AF.Exp)
            esum_g = misc.tile([GROUP, 1], F32, tag="esum")
            nc.vector.tensor_reduce(esum_g[:], e_g[:],
                                    axis=mybir.AxisListType.X, op=ALU.add)
            nc.sync.dma_start(esum_d.ap()[g0:g0 + GROUP, :], esum_g[:])
            erep = pa.tile([PROWS, S], F32, tag="pa")
            nc.tensor.matmul(
                erep[:], sel4[:, 0:PROWS], e_g[:], start=True, stop=True,
                skip_group_check=True)
            PT_sb = misc.tile([PROWS, S], F32, tag="ptsb")
            nc.vector.tensor_copy(PT_sb[:], PT[:, g2 * S:(g2 + 1) * S])
            prod = misc.tile([PROWS, S], F32, tag="prod")
            nc.vector.tensor_mul(prod[:], PT_sb[:], erep[:])
            nc.vector.tensor_reduce(numer[:, g2:g2 + 1], prod[:],
                                    axis=mybir.AxisListType.X, op=ALU.add)

        pending = []
        for r in range(ROUNDS):
            at = []
            for h in range(2):
                a = allp.tile([128, 4 * S], BF16, tag="allp")
                nc.sync.dma_start(a[:], allT[h].ap()[:, r * 4 * S:(r + 1) * 4 * S])
                at.append(a)
            pas = {}
            for u in range(2):
                for k in range(2):
                    paT = pa.tile([128, 2 * S], F32, tag="pa")
                    for h in range(2):
                        nc.tensor.matmul(
                            paT[:], UT_ap(h, k), at[h][:, u * 2 * S:(u + 1) * 2 * S],
                            start=(h == 0), stop=(h == 1))
                    pas[(u, k)] = paT
            tts = {}
            for u in range(2):
                for k in range(2):
                    tt = tpool.tile([128, 2 * S], BF16, tag="tpool")
                    for bb in range(2):
                        b = 4 * r + 2 * u + bb
                        nc.scalar.activation(
                            tt[:, bb * S:(bb + 1) * S],
                            pas[(u, k)][:, bb * S:(bb + 1) * S],
                            AF.Tanh, bias=lt[:, k * b_loc + b:k * b_loc + b + 1])
                    tts[(u, k)] = tt
            pending.append((r, at, tts))
            if len(pending) > 2:
                emit_tail(*pending.pop(0))
            if r == (3 * ROUNDS) // 4 and ROUNDS >= 4:
                endgame(0)
        for p in pending:
            emit_tail(*p)
        if not (ROUNDS >= 4):
            endgame(0)
        endgame(1)
        nc.sync.dma_start(numer_d.ap(), numer[:])
    nc.compile()
    return nc


def prep_core_inputs(all_c, last_c, U, W, V, MetaW, b_loc=B_LOC):
    GROUP = b_loc // 2
    x = np.ascontiguousarray(all_c.transpose(2, 0, 1)).astype(NBF)  # [H, b, S]
    m = {}
    m["allT0"] = np.ascontiguousarray(x[:128].reshape(128, b_loc * S))
    m["allT1"] = np.ascontiguousarray(x[128:].reshape(128, b_loc * S))
    l = (last_c @ W.T).astype(np.float32)
    m["LT"] = np.ascontiguousarray(
        l.T.reshape(2, 128, b_loc).transpose(1, 0, 2).reshape(128, 2 * b_loc))
    ut = U.reshape(2, 128, 2, 128).transpose(3, 2, 0, 1).reshape(128, 512)
    v = V[:, 0].reshape(2, 128).T
    mwp = np.zeros((128, 2, 8, 32), np.float32)
    for h in range(2):
        for i in range(8):
            mwp[:, h, i, 4 * i:4 * i + 4] = MetaW[:, 128 * h:128 * (h + 1)].T
    mwp = mwp.reshape(128, 512)
    sel4 = np.zeros((128, 128), np.float32)
    for mm in range(4 * GROUP):
        sel4[mm // 4, mm] = 1.0
    m["CBa"] = np.ascontiguousarray(ut).astype(NBF)
    m["CBb"] = np.ascontiguousarray(
        np.concatenate([v, mwp, sel4], axis=1)).astype(NBF)
    return m


def postprocess_core(numer, esum, Metab, b_loc=B_LOC):
    GROUP = b_loc // 2
    out = np.empty((b_loc, 4), np.float32)
    for g2 in range(2):
        out[g2 * GROUP:(g2 + 1) * GROUP] = numer[:4 * GROUP, g2].reshape(GROUP, 4)
    return out / esum.reshape(b_loc, 1) + Metab.reshape(1, 4)


_cache = {}


def _get_nc():
    if "nc" not in _cache:
        _cache["nc"] = build_nc(B_LOC)
    return _cache["nc"]


def kernel(all_memory, last_memory, U, W, V, MetaW, Metab):
    all_memory = np.asarray(all_memory, dtype=np.float32)
    last_memory = np.asarray(last_memory, dtype=np.float32)
    U = np.asarray(U, dtype=np.float32)
    W = np.asarray(W, dtype=np.float32)
    V = np.asarray(V, dtype=np.float32)
    MetaW = np.asarray(MetaW, dtype=np.float32)
    Metab = np.asarray(Metab, dtype=np.float32)
    nc = _get_nc()
    in_maps = []
    for c in range(N_CORES):
        sl = slice(c * B_LOC, (c + 1) * B_LOC)
        in_maps.append(prep_core_inputs(
            all_memory[sl], last_memory[sl], U, W, V, MetaW))
    res = run_bass_kernel_spmd(nc, in_maps, core_ids=list(range(N_CORES)))
    outs = [postprocess_core(res.results[c]["numer"], res.results[c]["esum"],
                             Metab) for c in range(N_CORES)]
    return np.concatenate(outs, axis=0).astype(np.float32)


# revision 6
# speedup vs baseline: 1.0207x; 1.0207x over previous
"""Additive-attention pooling (nn_Meta_Module) Trainium2 kernel.

Full inputs in, full output out. Internally: pure data-parallel over 8
NeuronCores (batch 512 -> 64/core). Per core, a Bass/Tile kernel computes
  a   = all_memory @ U.T            (PE, bf16, [k,(b,s)] layout)
  t   = tanh(a + last @ W.T)        (ScalarE, per-batch bias)
  sc  = V.T @ t                     (PE, V-stationary)
  P   = all_memory @ MetaW.T        (PE, col-tiled partition stack)
  out = (sum_s e^sc * P) / sum_s e^sc + Metab   (PE selector + DVE, host divide)
"""
import numpy as np
import ml_dtypes
from contextlib import ExitStack

import concourse.bass as bass
import concourse.tile as tile
import concourse.mybir as mybir
from concourse import bacc
from concourse.bass_utils import run_bass_kernel_spmd

BF16 = mybir.dt.bfloat16
F32 = mybir.dt.float32
AF = mybir.ActivationFunctionType
ALU = mybir.AluOpType
NBF = ml_dtypes.bfloat16

B, S, H = 512, 200, 256
N_CORES = 8
B_LOC = B // N_CORES


def build_nc(b_loc=B_LOC, debug=False):
    GROUP = b_loc // 2
    ROUNDS = b_loc // 4
    PROWS = 32 * ((GROUP + 7) // 8)
    nc = bacc.Bacc("TRN2", target_bir_lowering=False, debug=debug)

    def din(name, shape, dt=BF16):
        return nc.dram_tensor(name, shape, dt, kind="ExternalInput")

    allT = [din(f"allT{h}", [128, b_loc * S]) for h in range(2)]
    CBa_d = din("CBa", [128, 512])
    CBb_d = din("CBb", [128, 2 + 512 + 128])
    LT_d = din("LT", [128, 2 * b_loc], F32)
    numer_d = nc.dram_tensor("numer", [PROWS, 2], F32, kind="ExternalOutput")
    esum_d = nc.dram_tensor("esum", [b_loc, 1], F32, kind="ExternalOutput")

    with tile.TileContext(nc) as tc, ExitStack() as ctx:
        consts = ctx.enter_context(tc.tile_pool(name="consts", bufs=1))
        allp = ctx.enter_context(tc.tile_pool(name="allp", bufs=12))
        tpool = ctx.enter_context(tc.tile_pool(name="tpool", bufs=14))
        misc = ctx.enter_context(tc.tile_pool(name="misc", bufs=2))
        pa = ctx.enter_context(tc.tile_pool(name="pa", bufs=7, space="PSUM"))
        ps_pt = ctx.enter_context(tc.tile_pool(name="ps_pt", bufs=1, space="PSUM"))

        def load_const(dram, shape, tag, dt=BF16):
            t = consts.tile(shape, dt, tag=tag)
            nc.scalar.dma_start(t[:], dram.ap())
            return t

        cba = load_const(CBa_d, [128, 512], "c_cba")
        lt = load_const(LT_d, [128, 2 * b_loc], "c_lt", F32)
        cbb = load_const(CBb_d, [128, 2 + 512 + 128], "c_cbb")
        ut = cba[:, 0:512]
        vt = cbb[:, 0:2]
        mwp = cbb[:, 2:514]
        sel4 = cbb[0:GROUP, 514:642]

        def UT_ap(h, k):
            return ut[:, (2 * h + k) * 128:(2 * h + k + 1) * 128]

        def MW_ap(h, i):
            return mwp[:, (h * 8 + i) * 32:(h * 8 + i) * 32 + 32]

        scores_flat = consts.tile([1, b_loc * S], F32, tag="c_scf")
        PT = ps_pt.tile([PROWS, 2 * S], F32)
        pt_blocks = set()

        def emit_tail(r, at, tts):
            for u in range(2):
                sc = pa.tile([1, 2 * S], F32, tag="pa")
                for k in range(2):
                    nc.tensor.matmul(sc[:], vt[:, k:k + 1], tts[(u, k)][:],
                                     start=(k == 0), stop=(k == 1))
                nc.vector.tensor_copy(
                    scores_flat[0:1, (4 * r + 2 * u) * S:(4 * r + 2 * u + 2) * S],
                    sc[:])
            for u in range(2):
                for bb in range(2):
                    b = 4 * r + 2 * u + bb
                    bl = b % GROUP
                    g2 = b // GROUP
                    j32 = 32 * (bl // 8)
                    i8 = bl % 8
                    blk_new = (g2, j32) not in pt_blocks
                    pt_blocks.add((g2, j32))
                    for h in range(2):
                        nc.tensor.matmul(
                            PT[j32:j32 + 32, g2 * S:(g2 + 1) * S],
                            MW_ap(h, i8),
                            at[h][:, (2 * u + bb) * S:(2 * u + bb + 1) * S],
                            tile_position=(0, j32),
                            start=(blk_new and h == 0), stop=(h == 1),
                            skip_group_check=True)

        numer = misc.tile([PROWS, 2], F32, tag="numer", bufs=1)

        def endgame(g2):
            g0 = g2 * GROUP
            scT_g = misc.tile([GROUP, S], F32, tag="scT")
            nc.sync.dma_start(scT_g[:],
                              scores_flat[0:1, g0 * S:(g0 + GROUP) * S])
            e_g = misc.tile([GROUP, S], BF16, tag="e")
            nc.scalar.activation(e_g[:], scT_g[:], AF.Exp)
            esum_g = misc.tile([GROUP, 1], F32, tag="esum")
            nc.vector.tensor_reduce(esum_g[:], e_g[:],
                                    axis=mybir.AxisListType.X, op=ALU.add)
            nc.sync.dma_start(esum_d.ap()[g0:g0 + GROUP, :], esum_g[:])
            erep = pa.tile([PROWS, S], F32, tag="pa")
            nc.tensor.matmul(
                erep[:], sel4[:, 0:PROWS], e_g[:], start=True, stop=True,
                skip_group_check=True)
            PT_sb = misc.tile([PROWS, S], F32, tag="ptsb")
            nc.vector.tensor_copy(PT_sb[:], PT[:, g2 * S:(g2 + 1) * S])
            prod = misc.tile([PROWS, S], F32, tag="prod")
            nc.vector.tensor_mul(prod[:], PT_sb[:], erep[:])
            nc.vector.tensor_reduce(numer[:, g2:g2 + 1], prod[:],
                                    axis=mybir.AxisListType.X, op=ALU.add)

        pending = []
        for r in range(ROUNDS):
            at = []
            for h in range(2):
                a = allp.tile([128, 4 * S], BF16, tag="allp")
                nc.sync.dma_start(a[:], allT[h].ap()[:, r * 4 * S:(r + 1) * 4 * S])
                at.append(a)
            pas = {}
            for u in range(2):
                for k in range(2):
                    paT = pa.tile([128, 2 * S], F32, tag="pa")
                    for h in range(2):
                        nc.tensor.matmul(
                            paT[:], UT_ap(h, k), at[h][:, u * 2 * S:(u + 1) * 2 * S],
                            start=(h == 0), stop=(h == 1))
                    pas[(u, k)] = paT
            tts = {}
            for u in range(2):
                for k in range(2):
                    tt = tpool.tile([128, 2 * S], BF16, tag="tpool")
                    for bb in range(2):
                        b = 4 * r + 2 * u + bb
                        nc.scalar.activation(
                            tt[:, bb * S:(bb + 1) * S],
                            pas[(u, k)][:, bb * S:(bb + 1) * S],
                            AF.Tanh, bias=lt[:, k * b_loc + b:k * b_loc + b + 1])
                    tts[(u, k)] = tt
            pending.append((r, at, tts))
            while len(pending) > (2 if r < ROUNDS - 3 else 1):
                emit_tail(*pending.pop(0))
            if r == (3 * ROUNDS) // 4 and ROUNDS >= 4:
                endgame(0)
        for p in pending:
            emit_tail(*p)
        if not (ROUNDS >= 4):
            endgame(0)
        endgame(1)
        nc.sync.dma_start(numer_d.ap(), numer[:])
    nc.compile()
    return nc


def prep_core_inputs(all_c, last_c, U, W, V, MetaW, b_loc=B_LOC):
    GROUP = b_loc // 2
    x = np.ascontiguousarray(all_c.transpose(2, 0, 1)).astype(NBF)  # [H, b, S]
    m = {}
    m["allT0"] = np.ascontiguousarray(x[:128].reshape(128, b_loc * S))
    m["allT1"] = np.ascontiguousarray(x[128:].reshape(128, b_loc * S))
    l = (last_c @ W.T).astype(np.float32)
    m["LT"] = np.ascontiguousarray(
        l.T.reshape(2, 128, b_loc).transpose(1, 0, 2).reshape(128, 2 * b_loc))
    ut = U.reshape(2, 128, 2, 128).transpose(3, 2, 0, 1).reshape(128, 512)
    v = V[:, 0].reshape(2, 128).T
    mwp = np.zeros((128, 2, 8, 32), np.float32)
    for h in range(2):
        for i in range(8):
            mwp[:, h, i, 4 * i:4 * i + 4] = MetaW[:, 128 * h:128 * (h + 1)].T
    mwp = mwp.reshape(128, 512)
    sel4 = np.zeros((128, 128), np.float32)
    for mm in range(4 * GROUP):
        sel4[mm // 4, mm] = 1.0
    m["CBa"] = np.ascontiguousarray(ut).astype(NBF)
    m["CBb"] = np.ascontiguousarray(
        np.concatenate([v, mwp, sel4], axis=1)).astype(NBF)
    return m


def postprocess_core(numer, esum, Metab, b_loc=B_LOC):
    GROUP = b_loc // 2
    out = np.empty((b_loc, 4), np.float32)
    for g2 in range(2):
        out[g2 * GROUP:(g2 + 1) * GROUP] = numer[:4 * GROUP, g2].reshape(GROUP, 4)
    return out / esum.reshape(b_loc, 1) + Metab.reshape(1, 4)


_cache = {}


def _get_nc():
    if "nc" not in _cache:
        _cache["nc"] = build_nc(B_LOC)
    return _cache["nc"]


def kernel(all_memory, last_memory, U, W, V, MetaW, Metab):
    all_memory = np.asarray(all_memory, dtype=np.float32)
    last_memory = np.asarray(last_memory, dtype=np.float32)
    U = np.asarray(U, dtype=np.float32)
    W = np.asarray(W, dtype=np.float32)
    V = np.asarray(V, dtype=np.float32)
    MetaW = np.asarray(MetaW, dtype=np.float32)
    Metab = np.asarray(Metab, dtype=np.float32)
    nc = _get_nc()
    in_maps = []
    for c in range(N_CORES):
        sl = slice(c * B_LOC, (c + 1) * B_LOC)
        in_maps.append(prep_core_inputs(
            all_memory[sl], last_memory[sl], U, W, V, MetaW))
    res = run_bass_kernel_spmd(nc, in_maps, core_ids=list(range(N_CORES)))
    outs = [postprocess_core(res.results[c]["numer"], res.results[c]["esum"],
                             Metab) for c in range(N_CORES)]
    return np.concatenate(outs, axis=0).astype(np.float32)
